# revision 1
# baseline (speedup 1.0000x reference)
"""CLSTMCell fused cell kernel for 8 Trainium2 NeuronCores.

Data-parallel over the batch: each of the 8 cores processes a 512-row batch
shard; the four (D,4U) kernels and biases are replicated to every core.

Math (per batch shard, D = U = 1024):
    zr = xr@R + xi@I + hr@Rr + hi@Ir + br          [512, 4096]
    zi = xi@R - xr@I + hi@Rr - hr@Ir + bi          [512, 4096]
    per gate g (i,f,c,o) and half (r from zr, i from zi):
        i,f,o -> hard_sigmoid(z) = clip(0.2 z + 0.5, 0, 1);  c~ -> tanh(z)
    c = f*c_tm1 + i*tanh(c~);  h = o*tanh(c)

Device layout: output columns (n) on SBUF partitions, batch on the free dim.
Each matmul takes a [128k, 128n] weight tile as the stationary operand and a
transposed-activation block [128k, 512b] as the moving operand at float32r
(full-rate fp32), accumulating zT[n0:n0+128, :] over the 32 k-blocks. The
zr/zi pair shares each loaded weight tile, amortizing LDWEIGHTS. One phase =
one 128-wide u-block: all four gate psums for both halves live in the 8 PSUM
banks, so the gate combine needs no cross-phase staging, and biases are
per-partition scalars. Host-side work is layout only (slice/transpose/
reshape); zi's -xr/-hr blocks are negated once on device.
"""

import sys

sys.path.insert(0, "/opt/trn_rl_repo")

import numpy as np

import concourse.bacc as bacc
import concourse.mybir as mybir
import concourse.tile as tile
from concourse.bass_utils import run_bass_kernel_spmd

N_CORES = 8
B, D, U = 4096, 1024, 1024
BS = B // N_CORES          # batch rows per core
P = 128                    # SBUF partitions
KB = (2 * D + 2 * U) // P  # 32 contraction blocks of 128
NJ = U // P                # 8 u-blocks (phases)
F32 = mybir.dt.float32
F32R = mybir.dt.float32r
ADD = mybir.AluOpType.add
MULT = mybir.AluOpType.mult
MIN = mybir.AluOpType.min
MAX = mybir.AluOpType.max
TANH = None  # set lazily (mybir.ActivationFunctionType.Tanh)

_CACHE = {}


def _build():
    nc = bacc.Bacc("TRN2", target_bir_lowering=False, debug=False,
                   num_devices=N_CORES)
    Tanh = mybir.ActivationFunctionType.Tanh

    din = {}
    for name in ("xrT", "xiT", "hrT", "hiT"):
        din[name] = nc.dram_tensor(name, [D, BS], F32R, kind="ExternalInput").ap()
    din["c_prevT"] = nc.dram_tensor("c_prevT", [2 * U, BS], F32,
                                    kind="ExternalInput").ap()
    din["wperm"] = nc.dram_tensor("wperm", [NJ * KB * P, 4 * P], F32R,
                                  kind="ExternalInput").ap()
    din["brT"] = nc.dram_tensor("brT", [P, KB], F32, kind="ExternalInput").ap()
    din["biT"] = nc.dram_tensor("biT", [P, KB], F32, kind="ExternalInput").ap()
    h_outT = nc.dram_tensor("h_outT", [2 * U, BS], F32, kind="ExternalOutput").ap()
    c_outT = nc.dram_tensor("c_outT", [2 * U, BS], F32, kind="ExternalOutput").ap()

    with tile.TileContext(nc) as tc:
        with (
            tc.tile_pool(name="acts", bufs=48) as acts,
            tc.tile_pool(name="bias", bufs=4) as bias_p,
            tc.tile_pool(name="wpool", bufs=20) as wpool,
            tc.tile_pool(name="cprev", bufs=6) as cpool,
            tc.tile_pool(name="gat", bufs=6) as gat_p,
            tc.tile_pool(name="tmp", bufs=6) as tmp_p,
            tc.tile_pool(name="outs", bufs=8) as out_p,
            tc.tile_pool(name="psum", bufs=8, space="PSUM") as psum_p,
        ):
            # --- resident transposed-activation blocks, loaded lazily -------
            act_tiles = {}   # (src_name, block) -> tile
            neg_tiles = {}

            def act(name, j):
                t = act_tiles.get((name, j))
                if t is None:
                    t = acts.tile([P, BS], F32R, tag="acts", name=f"{name}{j}")
                    nc.sync.dma_start(t[:], din[name][j * P:(j + 1) * P, :])
                    act_tiles[(name, j)] = t
                return t

            def nact(name, j):
                t = neg_tiles.get((name, j))
                if t is None:
                    t = acts.tile([P, BS], F32R, tag="acts", name=f"n{name}{j}")
                    nc.vector.tensor_scalar_mul(t[:], act(name, j)[:], -1.0)
                    neg_tiles[(name, j)] = t
                return t

            A_SRC = ("xrT", "xiT", "hrT", "hiT")   # zr moving blocks by k//8
            B_SRC = ("xiT", "xrT", "hiT", "hrT")   # zi moving blocks (neg on 1,3)

            def a_block(k):
                return act(A_SRC[k // 8], k % 8)

            def b_block(k):
                name = B_SRC[k // 8]
                if (k // 8) % 2 == 1:
                    return nact(name, k % 8)
                return act(name, k % 8)

            # --- per-partition bias tiles [128, 32]; col m = n-block index --
            # raw for the c~ gate; 0.2*b + 0.5 pre-folded for the hsig gates.
            # Emitted lazily (first combine) so startup DMAs aren't queued
            # behind them.
            braw, bhs = [], []

            def emit_bias():
                for name in ("brT", "biT"):
                    t = bias_p.tile([P, KB], F32, tag="bias",
                                    name=f"braw_{name}")
                    nc.sync.dma_start(t[:], din[name][:, :])
                    braw.append(t)
                    t2 = bias_p.tile([P, KB], F32, tag="bias",
                                     name=f"bhs_{name}")
                    nc.vector.tensor_scalar(t2[:], t[:], 0.2, 0.5, MULT, ADD)
                    bhs.append(t2)

            # prime the first few moving blocks so phase 0's thin opening
            # k-steps (3 matmuls each) don't run ahead of the DMA stream
            for kk in range(3):
                a_block(kk)
                b_block(kk)

            # --- main loop: one phase per 128-wide u-block ------------------
            for j in range(NJ):
                # psum groups: (gate, z) -> zT[g*U + j*128 : .. , :] (8 banks)
                ps = {(g, z): psum_p.tile([P, BS], F32, tag="ps",
                                          name=f"ps_{j}_{g}_{z}")
                      for g in range(4) for z in range(2)}
                # --- gate combine, per half (emitted via combine()) -------
                cps = {}

                def emit_cps(j=j):
                    for z in range(2):
                        rows0 = z * U + j * P
                        cp = cpool.tile([P, BS], F32, tag="cprev",
                                        name=f"cp_{j}_{z}")
                        nc.sync.dma_start(
                            cp[:], din["c_prevT"][rows0:rows0 + P, :])
                        cps[z] = cp

                tc2s = {}

                def combine_ci(z, j=j, ps=ps):
                    if not braw:
                        emit_bias()
                    rows0 = z * U + j * P
                    cp = cps[z]

                    def relugate(g):
                        # relu(0.2*z + (0.2*b + 0.5)) on ACT straight from
                        # PSUM; the min(.,1) rides the consuming DVE op
                        t = gat_p.tile([P, BS], F32, tag="gat",
                                       name=f"hs_{j}_{z}_{g}")
                        bia = bhs[z][:, g * NJ + j:g * NJ + j + 1]
                        nc.scalar.activation(
                            t[:], ps[(g, z)][:],
                            mybir.ActivationFunctionType.Relu,
                            bias=bia, scale=0.2)
                        return t

                    # c~ = tanh(z_c + b_c), bias applied inside the ACT op
                    tct = tmp_p.tile([P, BS], F32, tag="tmp",
                                     name=f"tct_{j}_{z}")
                    nc.scalar.activation(
                        tct[:], ps[(2, z)][:], Tanh,
                        bias=braw[z][:, 2 * NJ + j:2 * NJ + j + 1], scale=1.0)

                    f_t = relugate(1)
                    i_t = relugate(0)
                    # c = min(f,1)*c_prev + min(i,1)*tanh(c~)
                    t1 = tmp_p.tile([P, BS], F32, tag="tmp", name=f"t1_{j}_{z}")
                    nc.vector.scalar_tensor_tensor(
                        t1[:], f_t[:], 1.0, cp[:], MIN, MULT)
                    t2 = tmp_p.tile([P, BS], F32, tag="tmp", name=f"t2_{j}_{z}")
                    nc.vector.scalar_tensor_tensor(
                        t2[:], i_t[:], 1.0, tct[:], MIN, MULT)
                    cn = out_p.tile([P, BS], F32, tag="out", name=f"cn_{j}_{z}")
                    nc.vector.tensor_tensor(cn[:], t1[:], t2[:], ADD)
                    nc.sync.dma_start(c_outT[rows0:rows0 + P, :], cn[:])
                    tc2 = tmp_p.tile([P, BS], F32, tag="tmp", name=f"tc2_{j}_{z}")
                    nc.scalar.activation(tc2[:], cn[:], Tanh)
                    tc2s[z] = (tc2, relugate)

                def combine_o(z, j=j, ps=ps):
                    rows0 = z * U + j * P
                    tc2, relugate = tc2s[z]
                    if j == NJ - 1 and z == 1:
                        # kernel tail: half-batch chunks pipeline the ACT
                        # relu, DVE mul and h DMA instead of serializing
                        # three full-width ops after the last matmul
                        o_t = gat_p.tile([P, BS], F32, tag="gat",
                                         name=f"hsl_{j}_{z}")
                        hn = out_p.tile([P, BS], F32, tag="out",
                                        name=f"hn_{j}_{z}")
                        bia = bhs[z][:, 3 * NJ + j:3 * NJ + j + 1]
                        for h0 in (0, BS // 2):
                            sl = slice(h0, h0 + BS // 2)
                            nc.scalar.activation(
                                o_t[:, sl], ps[(3, z)][:, sl],
                                mybir.ActivationFunctionType.Relu,
                                bias=bia, scale=0.2)
                            nc.vector.scalar_tensor_tensor(
                                hn[:, sl], o_t[:, sl], 1.0, tc2[:, sl],
                                MIN, MULT)
                            nc.sync.dma_start(
                                h_outT[rows0:rows0 + P, sl], hn[:, sl])
                        return
                    o_t = relugate(3)
                    hn = out_p.tile([P, BS], F32, tag="out", name=f"hn_{j}_{z}")
                    nc.vector.scalar_tensor_tensor(
                        hn[:], o_t[:], 1.0, tc2[:], MIN, MULT)
                    nc.sync.dma_start(h_outT[rows0:rows0 + P, :], hn[:])

                # staggered k-sweeps per group class: f/c~/i run at lag
                # 0 (real) / 4 (imag); the o-gate groups trail at lag 8 / 12,
                # so after the very last matmul only the short o->h chain
                # remains, and each half's c-chain hides under later matmuls
                LAG, OLAG = 4, 8
                wts = {}
                for t in range(KB + OLAG + LAG):
                    if t < KB:
                        k = t
                        if j == 0:
                            a_block(k)  # first-use DMA just ahead of its step
                        wt = wpool.tile([P, 4 * P], F32R, tag="w",
                                        name=f"w_{j}_{k}")
                        row0 = (j * KB + k) * P
                        # first weight tiles issue from ACT's HWDGE so they
                        # aren't queued behind the priming act-DMAs on SP
                        weng = nc.scalar if (j == 0 and k < 2) else nc.sync
                        weng.dma_start(wt[:],
                                       din["wperm"][row0:row0 + P, :])
                        wts[k] = wt
                        am = a_block(k)[:]
                        for g in (0, 1, 2):
                            nc.tensor.matmul(ps[(g, 0)][:],
                                             wt[:, g * P:(g + 1) * P], am,
                                             start=(k == 0), stop=(k == KB - 1))
                    if LAG <= t < KB + LAG:
                        k = t - LAG
                        if j == 0:
                            b_block(k)
                        bm = b_block(k)[:]
                        wt = wts[k]
                        for g in (0, 1, 2):
                            nc.tensor.matmul(ps[(g, 1)][:],
                                             wt[:, g * P:(g + 1) * P], bm,
                                             start=(k == 0), stop=(k == KB - 1))
                    if OLAG <= t < KB + OLAG:
                        k = t - OLAG
                        wt = wts[k]
                        nc.tensor.matmul(ps[(3, 0)][:],
                                         wt[:, 3 * P:4 * P], a_block(k)[:],
                                         start=(k == 0), stop=(k == KB - 1))
                    if OLAG + LAG <= t < KB + OLAG + LAG:
                        k = t - OLAG - LAG
                        wt = wts.pop(k)
                        nc.tensor.matmul(ps[(3, 1)][:],
                                         wt[:, 3 * P:4 * P], b_block(k)[:],
                                         start=(k == 0), stop=(k == KB - 1))
                    if t == 2:
                        emit_cps()
                    if t == KB - 1:
                        combine_ci(0)
                    if t == KB + LAG - 1:
                        combine_ci(1)
                    if t == KB + OLAG - 1:
                        combine_o(0)
                combine_o(1)

    nc.compile()
    return nc


def _in_maps(inputs, h_tm1, c_tm1, wr, wi, wrr, wir, br, bi):
    brT = np.ascontiguousarray(br.reshape(KB, P).T)
    biT = np.ascontiguousarray(bi.reshape(KB, P).T)
    # wperm[j, k, p, g*128+c] = W_src(k)[(k%8)*128+p, g*1024+j*128+c]
    wall = np.stack([wr, wi, wrr, wir])        # [s, 1024, 4096]
    v = wall.reshape(4, 8, P, 4, NJ, P)        # [s, kr, p, g, j, c]
    wperm = np.ascontiguousarray(
        v.transpose(4, 0, 1, 2, 3, 5).reshape(NJ * KB * P, 4 * P))
    maps = []
    for c in range(N_CORES):
        rows = slice(c * BS, (c + 1) * BS)
        maps.append({
            "xrT": np.ascontiguousarray(inputs[rows, :D].T),
            "xiT": np.ascontiguousarray(inputs[rows, D:].T),
            "hrT": np.ascontiguousarray(h_tm1[rows, :U].T),
            "hiT": np.ascontiguousarray(h_tm1[rows, U:].T),
            "c_prevT": np.ascontiguousarray(c_tm1[rows].T),
            "wperm": wperm,
            "brT": brT, "biT": biT,
        })
    return maps


def kernel(inputs, h_tm1, c_tm1, real_kernel, imaginary_kernel,
           real_recurrent_kernel, imaginary_recurrent_kernel,
           real_bias, imaginary_bias):
    if "nc" not in _CACHE:
        _CACHE["nc"] = _build()
    nc = _CACHE["nc"]

    maps = _in_maps(
        np.ascontiguousarray(inputs, dtype=np.float32),
        np.ascontiguousarray(h_tm1, dtype=np.float32),
        np.ascontiguousarray(c_tm1, dtype=np.float32),
        np.ascontiguousarray(real_kernel, dtype=np.float32),
        np.ascontiguousarray(imaginary_kernel, dtype=np.float32),
        np.ascontiguousarray(real_recurrent_kernel, dtype=np.float32),
        np.ascontiguousarray(imaginary_recurrent_kernel, dtype=np.float32),
        np.ascontiguousarray(real_bias, dtype=np.float32),
        np.ascontiguousarray(imaginary_bias, dtype=np.float32),
    )
    res = run_bass_kernel_spmd(nc, maps, list(range(N_CORES)))
    h = np.concatenate(
        [res.results[c]["h_outT"].T for c in range(N_CORES)], axis=0)
    c = np.concatenate(
        [res.results[c]["c_outT"].T for c in range(N_CORES)], axis=0)
    return np.ascontiguousarray(h), np.ascontiguousarray(c)



# revision 4
# speedup vs baseline: 1.0507x; 1.0507x over previous
"""CLSTMCell fused cell kernel for 8 Trainium2 NeuronCores.

Data-parallel over the batch: each of the 8 cores processes a 512-row batch
shard; weights are replicated.

The pre-activations have complex-multiplication structure. With
a = [x_r h_r], b = [x_i h_i]  (each [512, 2048]) and stacked weights
Wr = [R; Rr], Wi = [I; Ir]  (each [2048, 4096]):
    zr = a @ Wr + b @ Wi + br
    zi = b @ Wr - a @ Wi + bi
Karatsuba 3-product form (25% less tensor work than the 4-product form):
    m1 = a @ Wr            (fp32r)
    q  = b @ Wi            (bf16 - the only reduced-precision product)
    m3 = (a+b) @ (Wr-Wi)   (fp32r)
    zr = m1 + q,  zi = m3 - m1 + q
Per gate g (i,f,c,o): i,f,o -> hard_sigmoid, c~ -> tanh, then
    c = f*c_prev + i*tanh(c~);  h = o*tanh(c)
(The first U output columns use zr's gates, the last U use zi's.)

Device layout: output columns on PSUM partitions, batch on the free dim.
Work is organized in 32 groups (8 column-phases x 4 gates); each group
accumulates three 16-step psum chains (m1/q/m3) from [128k,128n] stationary
weight tiles and [128k,512b] moving activation blocks, then a short
DVE/ACT combine drains the three banks into the gate activation. s = a+b
is computed on device from the quartered a/b tiles. All DMA descriptors
are >=2KB per partition line; weights stream per-group (2MB fp32 + 0.5MB
bf16), double-prefetched two groups ahead.
"""

import sys

sys.path.insert(0, "/opt/trn_rl_repo")

import ml_dtypes
import numpy as np

import concourse.bacc as bacc
import concourse.mybir as mybir
import concourse.tile as tile
from concourse.bass_utils import run_bass_kernel_spmd

N_CORES = 8
B, D, U = 4096, 1024, 1024
BS = B // N_CORES          # batch rows per core
P = 128                    # SBUF partitions
KB = (D + U) // P          # 16 contraction blocks of 128
NT = U // P                # 8 column-phases per gate
NGRP = NT * 4              # 32 (phase, gate) groups
QK = 4                     # act tiles span 4 k-blocks each
F32 = mybir.dt.float32
F32R = mybir.dt.float32r
BF16 = mybir.dt.bfloat16
ADD = mybir.AluOpType.add
SUB = mybir.AluOpType.subtract
MULT = mybir.AluOpType.mult
MIN = mybir.AluOpType.min
NPBF16 = ml_dtypes.bfloat16

_CACHE = {}


def _build():
    nc = bacc.Bacc("TRN2", target_bir_lowering=False, debug=False,
                   num_devices=N_CORES)
    Tanh = mybir.ActivationFunctionType.Tanh
    Relu = mybir.ActivationFunctionType.Relu

    din = {}
    din["aT"] = nc.dram_tensor("aT", [P, KB * BS], F32R,
                               kind="ExternalInput").ap()
    din["bT"] = nc.dram_tensor("bT", [P, KB * BS], BF16,
                               kind="ExternalInput").ap()
    din["wf"] = nc.dram_tensor("wf", [NGRP * P, KB * 2 * P], F32R,
                               kind="ExternalInput").ap()
    din["wq"] = nc.dram_tensor("wq", [NGRP * P, KB * P], BF16,
                               kind="ExternalInput").ap()
    din["c_prevT"] = nc.dram_tensor("c_prevT", [2 * U, BS], F32,
                                    kind="ExternalInput").ap()
    din["brT"] = nc.dram_tensor("brT", [P, NGRP // 1], F32,
                                kind="ExternalInput").ap()
    din["biT"] = nc.dram_tensor("biT", [P, NGRP // 1], F32,
                                kind="ExternalInput").ap()
    h_outT = nc.dram_tensor("h_outT", [2 * U, BS], F32,
                            kind="ExternalOutput").ap()
    c_outT = nc.dram_tensor("c_outT", [2 * U, BS], F32,
                            kind="ExternalOutput").ap()

    with tile.TileContext(nc) as tc:
        with (
            tc.tile_pool(name="apool", bufs=QK) as apool,
            tc.tile_pool(name="bpool", bufs=QK) as bpool,
            tc.tile_pool(name="spool", bufs=QK) as spool,
            tc.tile_pool(name="wfp", bufs=3) as wfp,
            tc.tile_pool(name="wqp", bufs=3) as wqp,
            tc.tile_pool(name="bias", bufs=4) as bias_p,
            tc.tile_pool(name="cprev", bufs=4) as cpool,
            tc.tile_pool(name="comb", bufs=7) as comb_p,
            tc.tile_pool(name="gat", bufs=11) as gat_p,
            tc.tile_pool(name="tmp", bufs=6) as tmp_p,
            tc.tile_pool(name="outs", bufs=4) as out_p,
            tc.tile_pool(name="psum", bufs=8, space="PSUM") as psum_p,
        ):
            QW = QK * BS  # columns per act quarter-tile

            # --- resident activation quarters; s = a + b on device --------
            a_q, b_q, s_q = [], [], []
            for j in range(KB // QK):
                at = apool.tile([P, QW], F32R, tag="a", name=f"a{j}")
                nc.sync.dma_start(at[:], din["aT"][:, j * QW:(j + 1) * QW])
                a_q.append(at)
                bt = bpool.tile([P, QW], BF16, tag="b", name=f"b{j}")
                nc.sync.dma_start(bt[:], din["bT"][:, j * QW:(j + 1) * QW])
                b_q.append(bt)

            def amov(k):
                return a_q[k // QK][:, (k % QK) * BS:(k % QK + 1) * BS]

            def bmov(k):
                return b_q[k // QK][:, (k % QK) * BS:(k % QK + 1) * BS]

            def smov(k):
                return s_q[k // QK][:, (k % QK) * BS:(k % QK + 1) * BS]

            # --- weight prefetch: 2 fp32 chunks + 1 bf16 chunk per group --
            wtiles = {}

            def fetch_w(gi, eng):
                wf_t = wfp.tile([P, KB * 2 * P], F32R, tag="wf",
                                name=f"wf{gi}")
                wq_t = wqp.tile([P, KB * P], BF16, tag="wq", name=f"wq{gi}")
                r0 = gi * P
                half = KB * P
                eng.dma_start(wf_t[:, :half], din["wf"][r0:r0 + P, :half])
                eng.dma_start(wf_t[:, half:], din["wf"][r0:r0 + P, half:])
                eng.dma_start(wq_t[:], din["wq"][r0:r0 + P, :])
                wtiles[gi] = (wf_t, wq_t)

            # first two groups' weights race ahead on the ACT engine's DGE
            fetch_w(0, nc.scalar)
            fetch_w(1, nc.scalar)

            for j in range(KB // QK):
                st = spool.tile([P, QW], F32R, tag="s", name=f"s{j}")
                nc.vector.tensor_tensor(st[:], a_q[j][:], b_q[j][:], ADD)
                s_q.append(st)

            # --- per-partition bias tiles [128, 32]; col = g*8 + t --------
            braw, bhs = [], []

            def emit_bias():
                for name in ("brT", "biT"):
                    t = bias_p.tile([P, NGRP], F32, tag="bias",
                                    name=f"braw_{name}")
                    nc.scalar.dma_start(t[:], din[name][:, :])
                    braw.append(t)
                    t2 = bias_p.tile([P, NGRP], F32, tag="bias",
                                     name=f"bhs_{name}")
                    nc.vector.tensor_scalar(t2[:], t[:], 0.2, 0.5, MULT, ADD)
                    bhs.append(t2)

            # --- main loop: 8 phases x 4 gates -----------------------------
            for t in range(NT):
                cps = {}
                for z in range(2):
                    cp = cpool.tile([P, BS], F32, tag="cprev",
                                    name=f"cp_{t}_{z}")
                    rows0 = z * U + t * P
                    nc.sync.dma_start(cp[:],
                                      din["c_prevT"][rows0:rows0 + P, :])
                    cps[z] = cp
                gacts = {}
                tc2s = {}
                for g in range(4):
                    gi = t * 4 + g
                    if gi + 2 < NGRP:
                        fetch_w(gi + 2, nc.sync)
                    wf_t, wq_t = wtiles.pop(gi)
                    m1 = psum_p.tile([P, BS], F32, tag="ps",
                                     name=f"m1_{gi}")
                    qp = psum_p.tile([P, BS], F32, tag="ps", name=f"q_{gi}")
                    m3 = psum_p.tile([P, BS], F32, tag="ps",
                                     name=f"m3_{gi}")
                    for k in range(KB):
                        st, sp = (k == 0), (k == KB - 1)
                        c0 = 2 * k * P
                        nc.tensor.matmul(m1[:], wf_t[:, c0:c0 + P], amov(k),
                                         start=st, stop=sp)
                        nc.tensor.matmul(qp[:], wq_t[:, k * P:(k + 1) * P],
                                         bmov(k), start=st, stop=sp)
                        nc.tensor.matmul(m3[:], wf_t[:, c0 + P:c0 + 2 * P],
                                         smov(k), start=st, stop=sp)
                    if not braw:
                        emit_bias()
                    # drain the three banks: zr = m1+q, zi = m3+q-m1
                    qs = comb_p.tile([P, BS], F32, tag="comb",
                                     name=f"qs_{gi}")
                    nc.vector.tensor_copy(qs[:], qp[:])
                    zr = comb_p.tile([P, BS], F32, tag="comb",
                                     name=f"zr_{gi}")
                    nc.vector.tensor_tensor(zr[:], m1[:], qs[:], ADD)
                    t0 = comb_p.tile([P, BS], F32, tag="comb",
                                     name=f"t0_{gi}")
                    nc.vector.tensor_tensor(t0[:], m3[:], qs[:], ADD)
                    zi = comb_p.tile([P, BS], F32, tag="comb",
                                     name=f"zi_{gi}")
                    nc.vector.tensor_tensor(zi[:], t0[:], m1[:], SUB)
                    col = g * NT + t
                    for z, zz in enumerate((zr, zi)):
                        gt = gat_p.tile([P, BS], F32, tag="gat",
                                        name=f"g_{gi}_{z}")
                        if g == 2:
                            nc.scalar.activation(
                                gt[:], zz[:], Tanh,
                                bias=braw[z][:, col:col + 1], scale=1.0)
                        else:
                            # relu(0.2*z + 0.2*b + 0.5); min(.,1) rides the
                            # consuming DVE op
                            nc.scalar.activation(
                                gt[:], zz[:], Relu,
                                bias=bhs[z][:, col:col + 1], scale=0.2)
                        gacts[(g, z)] = gt
                    if g == 2:
                        for z in range(2):
                            rows0 = z * U + t * P
                            t1 = tmp_p.tile([P, BS], F32, tag="tmp",
                                            name=f"t1_{t}_{z}")
                            nc.vector.scalar_tensor_tensor(
                                t1[:], gacts[(1, z)][:], 1.0, cps[z][:],
                                MIN, MULT)
                            t2 = tmp_p.tile([P, BS], F32, tag="tmp",
                                            name=f"t2_{t}_{z}")
                            nc.vector.scalar_tensor_tensor(
                                t2[:], gacts[(0, z)][:], 1.0,
                                gacts[(2, z)][:], MIN, MULT)
                            cn = out_p.tile([P, BS], F32, tag="out",
                                            name=f"cn_{t}_{z}")
                            nc.vector.tensor_tensor(cn[:], t1[:], t2[:], ADD)
                            nc.gpsimd.dma_start(
                                c_outT[rows0:rows0 + P, :], cn[:])
                            tc2 = tmp_p.tile([P, BS], F32, tag="tmp",
                                             name=f"tc2_{t}_{z}")
                            nc.scalar.activation(tc2[:], cn[:], Tanh)
                            tc2s[z] = tc2
                    if g == 3:
                        for z in range(2):
                            rows0 = z * U + t * P
                            if t == NT - 1 and z == 1:
                                # kernel tail: half-batch chunks pipeline
                                # the final mul + DMA
                                hn = out_p.tile([P, BS], F32, tag="out",
                                                name=f"hn_{t}_{z}")
                                for h0 in (0, BS // 2):
                                    sl = slice(h0, h0 + BS // 2)
                                    nc.vector.scalar_tensor_tensor(
                                        hn[:, sl], gacts[(3, z)][:, sl],
                                        1.0, tc2s[z][:, sl], MIN, MULT)
                                    nc.gpsimd.dma_start(
                                        h_outT[rows0:rows0 + P, sl],
                                        hn[:, sl])
                                continue
                            hn = out_p.tile([P, BS], F32, tag="out",
                                            name=f"hn_{t}_{z}")
                            nc.vector.scalar_tensor_tensor(
                                hn[:], gacts[(3, z)][:], 1.0, tc2s[z][:],
                                MIN, MULT)
                            nc.gpsimd.dma_start(
                                h_outT[rows0:rows0 + P, :], hn[:])

    nc.compile()
    return nc


def _in_maps(inputs, h_tm1, c_tm1, wr, wi, wrr, wir, br, bi):
    Wr = np.vstack([wr, wrr])            # [2048, 4096]
    Wi = np.vstack([wi, wir])
    Wd = Wr - Wi

    def perm(W):  # [2048, 4096] -> [t, g, p, k, c]
        return W.reshape(KB, P, 4, NT, P).transpose(3, 2, 1, 0, 4)

    wf = np.stack([perm(Wr), perm(Wd)], axis=4)      # [t,g,p,k,2,c]
    wf = np.ascontiguousarray(
        wf.reshape(NGRP * P, KB * 2 * P), dtype=np.float32)
    wq = np.ascontiguousarray(
        perm(Wi).reshape(NGRP * P, KB * P)).astype(NPBF16)
    brT = np.ascontiguousarray(
        br.reshape(4, NT, P).transpose(2, 0, 1).reshape(P, NGRP))
    biT = np.ascontiguousarray(
        bi.reshape(4, NT, P).transpose(2, 0, 1).reshape(P, NGRP))

    def actperm(m, dt):  # [512, 2048] -> [128, 16*512], part = k-part
        v = m.T.reshape(KB, P, BS).transpose(1, 0, 2).reshape(P, KB * BS)
        return np.ascontiguousarray(v).astype(dt)

    maps = []
    for c in range(N_CORES):
        rows = slice(c * BS, (c + 1) * BS)
        a = np.hstack([inputs[rows, :D], h_tm1[rows, :U]])
        b = np.hstack([inputs[rows, D:], h_tm1[rows, U:]])
        maps.append({
            "aT": actperm(a, np.float32),
            "bT": actperm(b, NPBF16),
            "c_prevT": np.ascontiguousarray(c_tm1[rows].T),
            "wf": wf, "wq": wq,
            "brT": brT, "biT": biT,
        })
    return maps


def kernel(inputs, h_tm1, c_tm1, real_kernel, imaginary_kernel,
           real_recurrent_kernel, imaginary_recurrent_kernel,
           real_bias, imaginary_bias):
    if "nc" not in _CACHE:
        _CACHE["nc"] = _build()
    nc = _CACHE["nc"]

    maps = _in_maps(
        np.ascontiguousarray(inputs, dtype=np.float32),
        np.ascontiguousarray(h_tm1, dtype=np.float32),
        np.ascontiguousarray(c_tm1, dtype=np.float32),
        np.ascontiguousarray(real_kernel, dtype=np.float32),
        np.ascontiguousarray(imaginary_kernel, dtype=np.float32),
        np.ascontiguousarray(real_recurrent_kernel, dtype=np.float32),
        np.ascontiguousarray(imaginary_recurrent_kernel, dtype=np.float32),
        np.ascontiguousarray(real_bias, dtype=np.float32),
        np.ascontiguousarray(imaginary_bias, dtype=np.float32),
    )
    res = run_bass_kernel_spmd(nc, maps, list(range(N_CORES)))
    h = np.concatenate(
        [res.results[c]["h_outT"].T for c in range(N_CORES)], axis=0)
    c = np.concatenate(
        [res.results[c]["c_outT"].T for c in range(N_CORES)], axis=0)
    return np.ascontiguousarray(h), np.ascontiguousarray(c)


# revision 9
# speedup vs baseline: 1.2742x; 1.2128x over previous
"""CLSTMCell fused cell kernel for 8 Trainium2 NeuronCores.

Data-parallel over the batch: each of the 8 cores processes a 512-row batch
shard; weights are replicated.

The pre-activations have complex-multiplication structure. With
a = [x_r h_r], b = [x_i h_i]  (each [512, 2048]) and stacked weights
Wr = [R; Rr], Wi = [I; Ir]  (each [2048, 4096]):
    zr = a @ Wr + b @ Wi + br
    zi = b @ Wr - a @ Wi + bi
Karatsuba 3-product form (25% less tensor work than the 4-product form):
    m1 = a @ Wr            (fp32r)
    q  = b @ Wi            (bf16 - the only reduced-precision product)
    m3 = (a+b) @ (Wr-Wi)   (fp32r)
    zr = m1 + q,  zi = m3 - m1 + q
Per gate g (i,f,c,o): i,f,o -> hard_sigmoid, c~ -> tanh, then
    c = f*c_prev + i*tanh(c~);  h = o*tanh(c)
(The first U output columns use zr's gates, the last U use zi's.)

Device layout: output columns on PSUM partitions, batch on the free dim.
Work is organized in 32 groups (8 column-phases x 4 gates); each group
accumulates three 16-step psum chains (m1/q/m3) from [128k,128n] stationary
weight tiles and [128k,512b] moving activation blocks, then a short
DVE/ACT combine drains the three banks into the gate activation. s = a+b
is computed on device from the quartered a/b tiles. All DMA descriptors
are >=2KB per partition line; weights stream per-group (2MB fp32 + 0.5MB
bf16), double-prefetched two groups ahead.
"""

import sys

sys.path.insert(0, "/opt/trn_rl_repo")

import ml_dtypes
import numpy as np

import concourse.bacc as bacc
import concourse.mybir as mybir
import concourse.tile as tile
from concourse.bass_utils import run_bass_kernel_spmd

N_CORES = 8
B, D, U = 4096, 1024, 1024
BS = B // N_CORES          # batch rows per core
P = 128                    # SBUF partitions
KB = (D + U) // P          # 16 contraction blocks of 128
NT = U // P                # 8 column-phases per gate
NGRP = NT * 4              # 32 (phase, gate) groups
QK = 2                     # act tiles span 2 k-blocks each
F32 = mybir.dt.float32
F32R = mybir.dt.float32r
BF16 = mybir.dt.bfloat16
ADD = mybir.AluOpType.add
SUB = mybir.AluOpType.subtract
MULT = mybir.AluOpType.mult
MIN = mybir.AluOpType.min
NPBF16 = ml_dtypes.bfloat16

_CACHE = {}


def _build():
    nc = bacc.Bacc("TRN2", target_bir_lowering=False, debug=False,
                   num_devices=N_CORES)
    Tanh = mybir.ActivationFunctionType.Tanh
    Relu = mybir.ActivationFunctionType.Relu

    din = {}
    din["aT"] = nc.dram_tensor("aT", [P, KB * BS], F32R,
                               kind="ExternalInput").ap()
    din["bT"] = nc.dram_tensor("bT", [P, KB * BS], BF16,
                               kind="ExternalInput").ap()
    din["wf"] = nc.dram_tensor("wf", [NGRP * P, KB * 2 * P], F32R,
                               kind="ExternalInput").ap()
    din["wq"] = nc.dram_tensor("wq", [NGRP * P, KB * P], BF16,
                               kind="ExternalInput").ap()
    din["c_prevT"] = nc.dram_tensor("c_prevT", [2 * U, BS], F32,
                                    kind="ExternalInput").ap()
    din["brT"] = nc.dram_tensor("brT", [P, NGRP // 1], F32,
                                kind="ExternalInput").ap()
    din["biT"] = nc.dram_tensor("biT", [P, NGRP // 1], F32,
                                kind="ExternalInput").ap()
    h_outT = nc.dram_tensor("h_outT", [2 * U, BS], F32,
                            kind="ExternalOutput").ap()
    c_outT = nc.dram_tensor("c_outT", [2 * U, BS], F32,
                            kind="ExternalOutput").ap()

    with tile.TileContext(nc) as tc:
        with (
            tc.tile_pool(name="apool", bufs=KB // QK) as apool,
            tc.tile_pool(name="bpool", bufs=KB // QK) as bpool,
            tc.tile_pool(name="spool", bufs=KB // QK) as spool,
            tc.tile_pool(name="wfp", bufs=6) as wfp,
            tc.tile_pool(name="wqp", bufs=3) as wqp,
            tc.tile_pool(name="bias", bufs=4) as bias_p,
            tc.tile_pool(name="cprev", bufs=4) as cpool,
            tc.tile_pool(name="comb", bufs=7) as comb_p,
            tc.tile_pool(name="gat", bufs=11) as gat_p,
            tc.tile_pool(name="tmp", bufs=6) as tmp_p,
            tc.tile_pool(name="outs", bufs=4) as out_p,
            tc.tile_pool(name="psum", bufs=8, space="PSUM") as psum_p,
        ):
            QW = QK * BS  # columns per act chunk-tile

            # --- resident activation chunks; s = a + b on device ----------
            # b (the bf16 q-operand) streams first: the q chain opens every
            # group, so its 1MB prefix gates kernel start.
            a_q, b_q, s_q = [], [], []
            for j in range(KB // QK):
                bt = bpool.tile([P, QW], BF16, tag="b", name=f"b{j}")
                nc.sync.dma_start(bt[:], din["bT"][:, j * QW:(j + 1) * QW])
                b_q.append(bt)
            for j in range(KB // QK):
                at = apool.tile([P, QW], F32R, tag="a", name=f"a{j}")
                nc.sync.dma_start(at[:], din["aT"][:, j * QW:(j + 1) * QW])
                a_q.append(at)

            def amov(k):
                return a_q[k // QK][:, (k % QK) * BS:(k % QK + 1) * BS]

            def bmov(k):
                return b_q[k // QK][:, (k % QK) * BS:(k % QK + 1) * BS]

            def smov(k):
                return s_q[k // QK][:, (k % QK) * BS:(k % QK + 1) * BS]

            # --- weight prefetch: 2 fp32 tiles + 1 bf16 tile per group ----
            wtiles = {}

            def fetch_w(gi, eng):
                wq_t = wqp.tile([P, KB * P], BF16, tag="wq", name=f"wq{gi}")
                wfa = wfp.tile([P, KB * P], F32R, tag="wf", name=f"wfa{gi}")
                wfb = wfp.tile([P, KB * P], F32R, tag="wf", name=f"wfb{gi}")
                r0 = gi * P
                half = KB * P
                eng.dma_start(wq_t[:], din["wq"][r0:r0 + P, :])
                eng.dma_start(wfa[:], din["wf"][r0:r0 + P, :half])
                eng.dma_start(wfb[:], din["wf"][r0:r0 + P, half:])
                wtiles[gi] = (wfa, wfb, wq_t)

            # first two groups' weights race ahead on the ACT engine's DGE
            fetch_w(0, nc.scalar)
            fetch_w(1, nc.scalar)

            for j in range(KB // QK):
                st = spool.tile([P, QW], F32R, tag="s", name=f"s{j}")
                nc.vector.tensor_tensor(st[:], a_q[j][:], b_q[j][:], ADD)
                s_q.append(st)

            # --- per-partition bias tiles [128, 32]; col = g*8 + t --------
            braw, bhs = [], []

            def emit_bias():
                for name in ("brT", "biT"):
                    t = bias_p.tile([P, NGRP], F32, tag="bias",
                                    name=f"braw_{name}")
                    nc.scalar.dma_start(t[:], din[name][:, :])
                    braw.append(t)
                    t2 = bias_p.tile([P, NGRP], F32, tag="bias",
                                     name=f"bhs_{name}")
                    nc.vector.tensor_scalar(t2[:], t[:], 0.2, 0.5, MULT, ADD)
                    bhs.append(t2)

            # --- main loop: 8 phases x 4 gates -----------------------------
            for t in range(NT):
                cps = {}
                for z in range(2):
                    cp = cpool.tile([P, BS], F32, tag="cprev",
                                    name=f"cp_{t}_{z}")
                    rows0 = z * U + t * P
                    nc.sync.dma_start(cp[:],
                                      din["c_prevT"][rows0:rows0 + P, :])
                    cps[z] = cp
                gacts = {}
                tc2s = {}
                for g in range(4):
                    gi = t * 4 + g
                    if gi + 2 < NGRP:
                        fetch_w(gi + 2, nc.sync)
                    wfa, wfb, wq_t = wtiles.pop(gi)
                    m1 = psum_p.tile([P, BS], F32, tag="ps",
                                     name=f"m1_{gi}")
                    qp = psum_p.tile([P, BS], F32, tag="ps", name=f"q_{gi}")
                    m3 = psum_p.tile([P, BS], F32, tag="ps",
                                     name=f"m3_{gi}")

                    # one dtype-sequential chain per psum bank; even groups
                    # run q|m1|m3, odd groups m1|m3|q so consecutive groups
                    # keep the PE in the same precision mode at the seam
                    def chain_q():
                        for k in range(KB):
                            nc.tensor.matmul(
                                qp[:], wq_t[:, k * P:(k + 1) * P], bmov(k),
                                start=(k == 0), stop=(k == KB - 1))

                    def chain_m(ps, mov, off):
                        for k in range(KB):
                            wt = wfa if k < 8 else wfb
                            c0 = 2 * (k % 8) * P + off
                            nc.tensor.matmul(
                                ps[:], wt[:, c0:c0 + P], mov(k),
                                start=(k == 0), stop=(k == KB - 1))

                    if gi % 2 == 0:
                        chain_q()
                        chain_m(m1, amov, 0)
                        chain_m(m3, smov, P)
                    else:
                        chain_m(m1, amov, 0)
                        chain_m(m3, smov, P)
                        chain_q()
                    if not braw:
                        emit_bias()
                    # drain the three banks: zr = m1+q, zi = m3+q-m1
                    qs = comb_p.tile([P, BS], F32, tag="comb",
                                     name=f"qs_{gi}")
                    nc.vector.tensor_copy(qs[:], qp[:])
                    zr = comb_p.tile([P, BS], F32, tag="comb",
                                     name=f"zr_{gi}")
                    nc.vector.tensor_tensor(zr[:], m1[:], qs[:], ADD)
                    t0 = comb_p.tile([P, BS], F32, tag="comb",
                                     name=f"t0_{gi}")
                    nc.vector.tensor_tensor(t0[:], m3[:], qs[:], ADD)
                    zi = comb_p.tile([P, BS], F32, tag="comb",
                                     name=f"zi_{gi}")
                    nc.vector.tensor_tensor(zi[:], t0[:], m1[:], SUB)
                    col = g * NT + t
                    for z, zz in enumerate((zr, zi)):
                        gt = gat_p.tile([P, BS], F32, tag="gat",
                                        name=f"g_{gi}_{z}")
                        if g == 2:
                            nc.scalar.activation(
                                gt[:], zz[:], Tanh,
                                bias=braw[z][:, col:col + 1], scale=1.0)
                        else:
                            # relu(0.2*z + 0.2*b + 0.5); min(.,1) rides the
                            # consuming DVE op
                            nc.scalar.activation(
                                gt[:], zz[:], Relu,
                                bias=bhs[z][:, col:col + 1], scale=0.2)
                        gacts[(g, z)] = gt
                    if g == 2:
                        for z in range(2):
                            rows0 = z * U + t * P
                            t1 = tmp_p.tile([P, BS], F32, tag="tmp",
                                            name=f"t1_{t}_{z}")
                            nc.vector.scalar_tensor_tensor(
                                t1[:], gacts[(1, z)][:], 1.0, cps[z][:],
                                MIN, MULT)
                            t2 = tmp_p.tile([P, BS], F32, tag="tmp",
                                            name=f"t2_{t}_{z}")
                            nc.vector.scalar_tensor_tensor(
                                t2[:], gacts[(0, z)][:], 1.0,
                                gacts[(2, z)][:], MIN, MULT)
                            cn = out_p.tile([P, BS], F32, tag="out",
                                            name=f"cn_{t}_{z}")
                            nc.vector.tensor_tensor(cn[:], t1[:], t2[:], ADD)
                            nc.gpsimd.dma_start(
                                c_outT[rows0:rows0 + P, :], cn[:])
                            tc2 = tmp_p.tile([P, BS], F32, tag="tmp",
                                             name=f"tc2_{t}_{z}")
                            nc.scalar.activation(tc2[:], cn[:], Tanh)
                            tc2s[z] = tc2
                    if g == 3:
                        for z in range(2):
                            rows0 = z * U + t * P
                            if t == NT - 1 and z == 1:
                                # kernel tail: half-batch chunks pipeline
                                # the final mul + DMA
                                hn = out_p.tile([P, BS], F32, tag="out",
                                                name=f"hn_{t}_{z}")
                                for h0 in (0, BS // 2):
                                    sl = slice(h0, h0 + BS // 2)
                                    nc.vector.scalar_tensor_tensor(
                                        hn[:, sl], gacts[(3, z)][:, sl],
                                        1.0, tc2s[z][:, sl], MIN, MULT)
                                    nc.gpsimd.dma_start(
                                        h_outT[rows0:rows0 + P, sl],
                                        hn[:, sl])
                                continue
                            hn = out_p.tile([P, BS], F32, tag="out",
                                            name=f"hn_{t}_{z}")
                            nc.vector.scalar_tensor_tensor(
                                hn[:], gacts[(3, z)][:], 1.0, tc2s[z][:],
                                MIN, MULT)
                            nc.gpsimd.dma_start(
                                h_outT[rows0:rows0 + P, :], hn[:])

    nc.compile()
    return nc


def _in_maps(inputs, h_tm1, c_tm1, wr, wi, wrr, wir, br, bi):
    Wr = np.vstack([wr, wrr])            # [2048, 4096]
    Wi = np.vstack([wi, wir])
    Wd = Wr - Wi

    def perm(W):  # [2048, 4096] -> [t, g, p, k, c]
        return W.reshape(KB, P, 4, NT, P).transpose(3, 2, 1, 0, 4)

    wf = np.stack([perm(Wr), perm(Wd)], axis=4)      # [t,g,p,k,2,c]
    wf = np.ascontiguousarray(
        wf.reshape(NGRP * P, KB * 2 * P), dtype=np.float32)
    wq = np.ascontiguousarray(
        perm(Wi).reshape(NGRP * P, KB * P)).astype(NPBF16)
    brT = np.ascontiguousarray(
        br.reshape(4, NT, P).transpose(2, 0, 1).reshape(P, NGRP))
    biT = np.ascontiguousarray(
        bi.reshape(4, NT, P).transpose(2, 0, 1).reshape(P, NGRP))

    def actperm(m, dt):  # [512, 2048] -> [128, 16*512], part = k-part
        v = m.T.reshape(KB, P, BS).transpose(1, 0, 2).reshape(P, KB * BS)
        return np.ascontiguousarray(v).astype(dt)

    maps = []
    for c in range(N_CORES):
        rows = slice(c * BS, (c + 1) * BS)
        a = np.hstack([inputs[rows, :D], h_tm1[rows, :U]])
        b = np.hstack([inputs[rows, D:], h_tm1[rows, U:]])
        maps.append({
            "aT": actperm(a, np.float32),
            "bT": actperm(b, NPBF16),
            "c_prevT": np.ascontiguousarray(c_tm1[rows].T),
            "wf": wf, "wq": wq,
            "brT": brT, "biT": biT,
        })
    return maps


def kernel(inputs, h_tm1, c_tm1, real_kernel, imaginary_kernel,
           real_recurrent_kernel, imaginary_recurrent_kernel,
           real_bias, imaginary_bias):
    if "nc" not in _CACHE:
        _CACHE["nc"] = _build()
    nc = _CACHE["nc"]

    maps = _in_maps(
        np.ascontiguousarray(inputs, dtype=np.float32),
        np.ascontiguousarray(h_tm1, dtype=np.float32),
        np.ascontiguousarray(c_tm1, dtype=np.float32),
        np.ascontiguousarray(real_kernel, dtype=np.float32),
        np.ascontiguousarray(imaginary_kernel, dtype=np.float32),
        np.ascontiguousarray(real_recurrent_kernel, dtype=np.float32),
        np.ascontiguousarray(imaginary_recurrent_kernel, dtype=np.float32),
        np.ascontiguousarray(real_bias, dtype=np.float32),
        np.ascontiguousarray(imaginary_bias, dtype=np.float32),
    )
    res = run_bass_kernel_spmd(nc, maps, list(range(N_CORES)))
    h = np.concatenate(
        [res.results[c]["h_outT"].T for c in range(N_CORES)], axis=0)
    c = np.concatenate(
        [res.results[c]["c_outT"].T for c in range(N_CORES)], axis=0)
    return np.ascontiguousarray(h), np.ascontiguousarray(c)
